# revision 33
# baseline (speedup 1.0000x reference)
"""Trainium2 Bass kernel for BinarizedInputNetwork (v2).

Contract: kernel(**inputs) takes the FULL unsharded inputs (batch 128) and
returns the FULL [128, 12] float32 softmax output. Internally shards the
batch across 8 NeuronCores (16 images each), runs one SPMD Bass program.

Network (per image, input [1,128,128]):
  conv1 3x3 s2 p1 (1->64)  + BN + ReLU -> sign       => binary acts
  conv2 3x3 s1 p1 (64->128, sign wts)  + BN + ReLU -> sign
  conv3 3x3 s2 p1 (128->128, sign wts) + BN + ReLU -> sign
  conv4 3x3 s1 p1 (128->192, sign wts) + BN + ReLU -> sign
  conv5 1x1 s1 p0 (192->192, sign wts) + BN + ReLU
  conv6 1x1 (192->12) + b ; GAP ; FC 12x12 + b ; softmax

Device mapping (changes vs v1):
  - conv1 runs in fp32r (full-rate on PE vs 4x-slow fp32; ~2e-4 rel err,
    harmless before binarization).
  - Binary activations use TWO exact encodings so the psum->sbuf binarize
    drains can be split across BOTH VectorE and ScalarE (v1 was DVE-bound):
      {0,2} = 2*a   via DVE tensor_scalar (is_gt t)*2
      {-1,1} = 2a-1 via ScalarE activation Sign(x - t)  (pads = -1)
    Thresholds/affines absorb the encoding exactly (integer arithmetic in
    fp8 stays exact): for +-1 inputs the next layer's threshold gains a
    -sum(w) correction; for {0,2} inputs thresholds just double.
    Assignment: A1 {0,2} (DVE), A2 +-1 (ACT), A3 {0,2} (DVE),
    A4a +-1 (ACT), A4b {0,2} (DVE), L5 relu+GAP-accum on ACT.
  - PSUM tiles span 2 banks ([*,1024]) so each drain instruction covers
    1024 columns (half the per-instruction overhead of v1's 512).
  - Every fp8 matmul uses DoubleRow; former solo taps get a zero-weighted
    second j slot (0.5 vs 1.0 PE cycles per output column).
  - conv1 matmuls for the NEXT image pair are interleaved into the PE
    stream between L2 chunks so slow c1 drains never head-block the PE.
"""

import sys

sys.path.insert(0, "/opt/trn_rl_repo")

import numpy as np

import concourse.ap as apm
import concourse.bass as bass
import concourse.mybir as mybir
import concourse.bacc as bacc
import concourse.tile as tile
from concourse.bass_utils import run_bass_kernel_spmd

F32 = mybir.dt.float32
F32R = mybir.dt.float32r
FP8 = mybir.dt.float8e4
AX = mybir.AxisListType
OP = mybir.AluOpType
ACT = mybir.ActivationFunctionType
DR = mybir.MatmulPerfMode.DoubleRow

N_CORES = 8
B = 16  # images per core
NPAIR = B // 2

EPS = 1e-5

# geometry
H1, W1 = 64, 64          # conv1 output spatial
P1 = W1 + 2              # padded width/height for A1 (66)
S1 = P1 * P1             # 4356
BO1 = 4368               # A1 shifted-copy block offset (16-aligned)
H3, W3 = 32, 32          # conv3 output spatial
P3 = W3 + 2              # 34
P2A = 80                 # A2 row pitch (66 rows x 80)
S2A = 66 * P2A           # 5280
P3A = 48                 # A3 row pitch (34 rows x 48)
S3A = P3 * P3A           # 1632
NPOS = H3 * W3           # 1024 valid positions for L5/GAP

_CACHE = {}


def _ap(base2d, off, dims):
    """Custom AP over an SBUF tile slice: base partition dim + free dims
    (supports overlapping patterns rearrange can't express)."""
    return apm.AP(tensor=base2d.tensor, offset=base2d.offset + off,
                  ap=[list(base2d.ap[0])] + [list(d) for d in dims])


def _build(reps=1, sim_compat=False):
    """Trace + compile the Bass program (cached). reps>1 replicates the whole
    pipeline on-device (for timing via wall-clock differencing).

    sim_compat=True lowers every DoubleRow matmul to two plain accumulating
    matmuls (identical math + AP offsets) because the exec interpreter only
    supports 3-dim DR rhs APs. Used by sim_test.py only."""
    key = f"nc{reps}" + ("s" if sim_compat else "")
    if key in _CACHE:
        return _CACHE

    nc = bacc.Bacc("TRN2", target_bir_lowering=False, debug=False,
                   num_devices=N_CORES)

    # ---- DRAM I/O ----
    # host-side im2col of the 1-channel input, split-precision for exact
    # fp32r matmul: rows 0-8 = X_hi (tf32-rounded), 9-17 = X_lo (residual,
    # tf32-rounded), 18-26 = X_hi again. Paired with lhsT rows
    # [W_hi; W_hi; W_lo] this computes W@X to ~2^-20 despite fp32r inputs,
    # at the same 1-cycle/row PE rate (cost is K-independent).
    dX = nc.dram_tensor("x", [B, 27, 4096], F32R, kind="ExternalInput").ap()
    # conv1 weights at partition rows 0-8 and 64-72 (2-way row tiling)
    dW1T = nc.dram_tensor("w1t4", [128, 64], F32R, kind="ExternalInput").ap()
    dW2D1 = nc.dram_tensor("w2d1", [128, 256], FP8, kind="ExternalInput").ap()
    dW2D2 = nc.dram_tensor("w2d2", [128, 256], FP8, kind="ExternalInput").ap()
    dW2D3 = nc.dram_tensor("w2d3", [64, 256], FP8, kind="ExternalInput").ap()
    dW3D = nc.dram_tensor("w3d", [128, 768], FP8, kind="ExternalInput").ap()
    dW3D3 = nc.dram_tensor("w3d3", [128, 256], FP8, kind="ExternalInput").ap()
    dW3S = nc.dram_tensor("w3s2", [128, 256], FP8, kind="ExternalInput").ap()
    dW4DA = nc.dram_tensor("w4da", [128, 768], FP8, kind="ExternalInput").ap()
    dW4D3A = nc.dram_tensor("w4d3a", [128, 256], FP8, kind="ExternalInput").ap()
    dW4SA = nc.dram_tensor("w4sa2", [128, 256], FP8, kind="ExternalInput").ap()
    dW4DB = nc.dram_tensor("w4db", [128, 384], FP8, kind="ExternalInput").ap()
    dW4D3B = nc.dram_tensor("w4d3b", [128, 128], FP8, kind="ExternalInput").ap()
    dW4SB = nc.dram_tensor("w4sb2", [128, 128], FP8, kind="ExternalInput").ap()
    dW5DA = nc.dram_tensor("w5da", [128, 256], FP8, kind="ExternalInput").ap()
    dW5DB = nc.dram_tensor("w5db", [128, 128], FP8, kind="ExternalInput").ap()
    dT1 = nc.dram_tensor("t1", [64, 1], F32, kind="ExternalInput").ap()
    dNT1 = nc.dram_tensor("nt1", [64, 1], F32, kind="ExternalInput").ap()
    dNT2 = nc.dram_tensor("nt2", [128, 1], F32, kind="ExternalInput").ap()
    dNT2P = nc.dram_tensor("nt2p", [128, 1], F32, kind="ExternalInput").ap()
    dT3 = nc.dram_tensor("t3", [128, 1], F32, kind="ExternalInput").ap()
    dNT4A = nc.dram_tensor("nt4a", [128, 1], F32, kind="ExternalInput").ap()
    dT4AD = nc.dram_tensor("t4ad", [128, 1], F32, kind="ExternalInput").ap()
    dT4B = nc.dram_tensor("t4b", [64, 1], F32, kind="ExternalInput").ap()
    dS1A = nc.dram_tensor("s1a", [128, 1], F32, kind="ExternalInput").ap()
    dS1B = nc.dram_tensor("s1b", [64, 1], F32, kind="ExternalInput").ap()
    dS1AO = nc.dram_tensor("s1ao", [128, 1], F32, kind="ExternalInput").ap()
    dS1BO = nc.dram_tensor("s1bo", [64, 1], F32, kind="ExternalInput").ap()
    dWTa = nc.dram_tensor("wta", [128, 12], F32, kind="ExternalInput").ap()
    dWTb = nc.dram_tensor("wtb", [65, 12], F32, kind="ExternalInput").ap()
    dY = nc.dram_tensor("y", [B, 12], F32, kind="ExternalOutput").ap()

    with tile.TileContext(nc) as tc:
        with tc.tile_pool(name="const", bufs=1) as cp, \
             tc.tile_pool(name="work", bufs=2) as wp, \
             tc.tile_pool(name="psA", bufs=2, space="PSUM") as pa, \
             tc.tile_pool(name="psC", bufs=2, space="PSUM") as pc:

            def ctile(name, shape, dtype):
                return cp.tile(shape, dtype, tag=name, name=name)

            # ---- persistent weight/param tiles ----
            cW1T = ctile("cW1T", [128, 64], F32R)
            cW2D1 = ctile("cW2D1", [128, 256], FP8)
            cW2D2 = ctile("cW2D2", [128, 256], FP8)
            cW2D3 = ctile("cW2D3", [64, 256], FP8)
            cW3D = ctile("cW3D", [128, 768], FP8)
            cW3D3 = ctile("cW3D3", [128, 256], FP8)
            cW3S = ctile("cW3S", [128, 256], FP8)
            cW4DA = ctile("cW4DA", [128, 768], FP8)
            cW4D3A = ctile("cW4D3A", [128, 256], FP8)
            cW4SA = ctile("cW4SA", [128, 256], FP8)
            cW4DB = ctile("cW4DB", [128, 384], FP8)
            cW4D3B = ctile("cW4D3B", [128, 128], FP8)
            cW4SB = ctile("cW4SB", [128, 128], FP8)
            cW5DA = ctile("cW5DA", [128, 256], FP8)
            cW5DB = ctile("cW5DB", [128, 128], FP8)
            cT1 = ctile("cT1", [64, 1], F32)
            cNT1 = ctile("cNT1", [64, 1], F32)
            cNT2 = ctile("cNT2", [128, 1], F32)
            cNT2P = ctile("cNT2P", [128, 1], F32)
            cT3 = ctile("cT3", [128, 1], F32)
            cNT4A = ctile("cNT4A", [128, 1], F32)
            cT4AD = ctile("cT4AD", [128, 1], F32)
            cT4B = ctile("cT4B", [64, 1], F32)
            cS1A = ctile("cS1A", [128, 1], F32)
            cS1B = ctile("cS1B", [64, 1], F32)
            cS1AO = ctile("cS1AO", [128, 1], F32)
            cS1BO = ctile("cS1BO", [64, 1], F32)
            cWTa = ctile("cWTa", [128, 12], F32)
            cWTb = ctile("cWTb", [65, 12], F32)

            # load-order matters at startup: conv1's deps go first on the
            # HWDGE (sync) queue so pair-0 conv1 starts right after the IC
            # DMA; L2 weights ride the Pool SWDGE path in parallel; the rest
            # (first needed several us in) go via the scalar engine's queue.
            for t_, d_ in [(cW1T, dW1T), (cT1, dT1), (cNT1, dNT1),
                           (cNT2, dNT2), (cNT2P, dNT2P)]:
                nc.sync.dma_start(t_[:], d_[:])
            for t_, d_ in [(cW2D1, dW2D1), (cW2D2, dW2D2), (cW2D3, dW2D3)]:
                nc.gpsimd.dma_start(t_[:], d_[:])
            for t_, d_ in [(cW3D, dW3D), (cW3D3, dW3D3), (cW3S, dW3S),
                           (cW4DA, dW4DA), (cW4D3A, dW4D3A), (cW4SA, dW4SA),
                           (cW4DB, dW4DB), (cW4D3B, dW4D3B),
                           (cW4SB, dW4SB), (cW5DA, dW5DA),
                           (cW5DB, dW5DB),
                           (cT3, dT3), (cNT4A, dNT4A), (cT4AD, dT4AD),
                           (cT4B, dT4B),
                           (cS1A, dS1A), (cS1B, dS1B),
                           (cS1AO, dS1AO), (cS1BO, dS1BO),
                           (cWTa, dWTa), (cWTb, dWTb)]:
                nc.scalar.dma_start(t_[:], d_[:])

            # ---- persistent activation buffers ----
            # IC: image pair (g=0 at partitions 0-8, g=1 at 64-72)
            IC = [ctile(f"IC{p}", [128, 4096], F32R) for p in range(2)]
            # A1 {0,2}: block0 [lo=img, hi=img row+1]; blk1 = +1; blk2 = +130
            A1 = [ctile(f"A1_{p}", [128, 3 * BO1], FP8) for p in range(4)]
            # A2 +-1 (pad -1), pitch 80; +1-col copy block at S2A
            A2 = [ctile(f"A2_{p}", [128, 2 * S2A], FP8) for p in range(2)]
            # A3 {0,2} (pad 0), pitch 48; +1-col copy block at S3A
            A3 = [ctile(f"A3_{p}", [128, 2 * S3A], FP8) for p in range(2)]
            # A4: block0 = couts 0-127 (+-1); block1 at NPOS = couts 128-191
            # {0,2} on partitions 0-63, zeros above
            A4 = [ctile(f"A4_{p}", [128, 2 * NPOS], FP8) for p in range(2)]
            MACCa = ctile("MACCa", [128, B], F32)
            MsumB = ctile("MsumB", [65, B], F32)
            SCRa = [ctile(f"SCRa{p}", [128, 1024], F32) for p in range(2)]
            SCRb = [ctile(f"SCRb{p}", [64, 1024], F32) for p in range(2)]

            # ---- one-time pad/guard memsets (pad-only, cheap) ----
            def a1_pads(p, val):
                """Pad regions of A1[p] that encode 'activation 0': 0.0 for
                {0,2} images, -1.0 for +-1 images (pair-0 image 1 only)."""
                a1 = A1[p]
                # row 0 + row 65 + tail of block0
                nc.gpsimd.memset(a1[:, 0:P1], val)
                nc.gpsimd.memset(a1[:, 65 * P1:BO1], val)
                # cols 65,0 pairs of rows 0..64: [[66,65],[1,2]] at offset 65
                nc.gpsimd.memset(_ap(a1[:, :], 65, [[P1, 65], [1, 2]]), val)

            for p in range(4):
                a1 = A1[p]
                a1_pads(p, -1.0 if p == 1 else 0.0)
                # blk copy tails never covered by the per-image copies;
                # blk2 hi half is only read zero-weighted, so it is copied
                # lo-only and memset here
                nc.gpsimd.memset(a1[:, BO1 + 4355:2 * BO1], 0.0)
                nc.gpsimd.memset(a1[0:64, 2 * BO1 + 4226:3 * BO1], 0.0)
                nc.gpsimd.memset(a1[64:128, 2 * BO1:3 * BO1], 0.0)
            for p in range(2):
                a2 = A2[p]
                nc.gpsimd.memset(a2[:, 0:P2A], -1.0)
                nc.gpsimd.memset(a2[:, 65 * P2A:S2A], -1.0)
                nc.gpsimd.memset(
                    _ap(a2[:, :], 65, [[P2A, 65], [1, 16]]), -1.0)
                # blk1: rows 0 and 65 + col gaps (copies only cover interior)
                nc.gpsimd.memset(a2[:, S2A:S2A + P2A], -1.0)
                nc.gpsimd.memset(a2[:, S2A + 65 * P2A:2 * S2A], -1.0)
                nc.gpsimd.memset(
                    _ap(a2[:, :], S2A + 64, [[P2A, 65], [1, 17]]), -1.0)
                a3 = A3[p]
                nc.gpsimd.memset(a3[:, 0:P3A], 0.0)
                nc.gpsimd.memset(a3[:, 33 * P3A:S3A], 0.0)
                nc.gpsimd.memset(
                    _ap(a3[:, :], 33, [[P3A, 33], [1, 16]]), 0.0)
                nc.gpsimd.memset(a3[:, S3A:S3A + P3A], 0.0)
                nc.gpsimd.memset(a3[:, S3A + 33 * P3A:2 * S3A], 0.0)
                nc.gpsimd.memset(
                    _ap(a3[:, :], S3A + 32, [[P3A, 33], [1, 17]]), 0.0)
                nc.gpsimd.memset(A4[p][64:128, NPOS:2 * NPOS], 0.0)
            nc.vector.memset(MsumB[64:65, :], 1.0)

            def mmdr(out, wflat, m, rhs_tile, off0, jstep, rest, start, stop,
                     zero_j1=False):
                """DoubleRow matmul: lhsT = wflat viewed [p, 2, m], rhs j-pair
                at (off0, off0+jstep). zero_j1: second slot is zero weights
                (rhs garbage, masked). sim_compat lowers to 2 plain matmuls."""
                if not sim_compat:
                    nc.tensor.matmul(
                        out, wflat.rearrange("p (j m) -> p j m", j=2),
                        _ap(rhs_tile, off0, [[jstep, 2]] + rest),
                        start=start, stop=stop, perf_mode=DR)
                    return
                nc.tensor.matmul(out, wflat[:, 0:m],
                                 _ap(rhs_tile, off0, rest),
                                 start=start, stop=stop and zero_j1)
                if not zero_j1:
                    nc.tensor.matmul(out, wflat[:, m:2 * m],
                                     _ap(rhs_tile, off0 + jstep, rest),
                                     start=False, stop=stop)

            # ---------------- layer bodies ------------------------------------
            def ic_load(k):
                """DMA im2col input for pair k."""
                pq = k % 2
                nc.sync.dma_start(IC[pq][0:27, :], dX[2 * k])
                nc.sync.dma_start(IC[pq][64:91, :], dX[2 * k + 1])

            c1_state = {}

            def c1_chunk(k, r):
                """conv1 r-chunk (8 out rows) for both images of pair k.
                Tiles [64,1024] hold 2 r-chunks; drain (DVE, {0,2}) + A1
                copies fire after the odd r."""
                pq = k % 2
                for g in range(2):
                    if r % 2 == 0:
                        c1_state[g] = pc.tile([64, 1024], F32, tag="c1",
                                              name=f"ps_c1_{k}_{r}_{g}")
                    ps = c1_state[g]
                    nc.tensor.matmul(
                        ps[:, (r % 2) * 512:(r % 2) * 512 + 512],
                        cW1T[64 * g:64 * g + 27, :],
                        IC[pq][64 * g:64 * g + 27, r * 512:(r + 1) * 512],
                        start=True, stop=True,
                        tile_position=(64 * g, 0))
                if r % 2 == 1:
                    y0 = 8 * (r - 1)
                    for g in range(2):
                        a1t = A1[(2 * k + g) % 4]
                        dst = _ap(a1t[0:64, :], (y0 + 1) * P1 + 1,
                                  [[P1, 16], [1, 64]])
                        src = c1_state[g][:, 0:1024].rearrange(
                            "p (a b) -> p a b", b=64)
                        if k == 0 and g == 1:
                            # pair 0 has no other work to hide its drains:
                            # split them across engines (image 1 -> ACT, +-1)
                            nc.scalar.activation(dst, src, ACT.Sign,
                                                 bias=cNT1[:], scale=1.0)
                        else:
                            nc.vector.tensor_scalar(dst, src, cT1[:], 2.0,
                                                    OP.is_gt, OP.mult)
                if r == 7:
                    for g in range(2):
                        a1t = A1[(2 * k + g) % 4]
                        # hi half: copy2[q] = copy1[q + P1]. Issued from the
                        # Pool engine (SWDGE) to keep the SP sequencer and
                        # shared HWDGE off the critical path.
                        nc.gpsimd.dma_start(a1t[64:128, 0:S1 - P1],
                                            a1t[0:64, P1:S1])
                        # shifted blocks for DoubleRow pair dim (blk2 is only
                        # read zero-weighted on the hi half: copy lo only)
                        nc.gpsimd.dma_start(a1t[:, BO1:BO1 + 4355],
                                            a1t[:, 1:4356])
                        nc.gpsimd.dma_start(a1t[0:64, 2 * BO1:2 * BO1 + 4226],
                                            a1t[0:64, 130:4356])

            l2_state = {}

            def l2_chunk(i, c):
                """L2 chunk c (8 out rows, valid cols only) for image i.
                Drain (ACT Sign, +-1) after odd c."""
                p = i % 2
                a1f = A1[i % 4][:, :]
                a1lo = A1[i % 4][0:64, :]
                y0 = 8 * c
                q0 = y0 * P1
                if c % 2 == 0:
                    l2_state[0] = pa.tile([128, 1024], F32, tag="mm",
                                          name=f"ps_l2_{i}_{c}")
                ps = l2_state[0]
                out = ps[:, (c % 2) * 512:(c % 2) * 512 + 512]
                rest = [[P1, 8], [1, 64]]
                mmdr(out, cW2D1[:], 128, a1f, q0, BO1, rest, True, False)
                mmdr(out, cW2D2[:], 128, a1f, q0 + 2, 2 * BO1, rest,
                     False, False)
                mmdr(out, cW2D3[:], 128, a1lo, q0 + 133, BO1, rest,
                     False, True)
                if c % 2 == 1:
                    yb = 8 * (c - 1)
                    dst = _ap(A2[p][:, :], (yb + 1) * P2A + 1,
                              [[P2A, 16], [1, 64]])
                    src = ps[:, 0:1024].rearrange("p (a b) -> p a b", b=64)
                    bias = cNT2P if i == 1 else cNT2  # image 1's A1 is +-1
                    nc.scalar.activation(dst, src, ACT.Sign, bias=bias[:],
                                         scale=1.0)
                    # +1-col shifted block for this row range
                    ql = (yb + 1) * P2A + 1
                    qh = (yb + 16) * P2A + 65
                    nc.sync.dma_start(
                        A2[p][:, S2A + ql - 1:S2A + qh - 1], A2[p][:, ql:qh])

            def l3_block(i):
                """L3 (stride 2): 2 matmul groups -> one [128,1024] DVE drain
                ({0,2})."""
                p = i % 2
                a2f = A2[p][:, :]
                ps = pa.tile([128, 1024], F32, tag="mm", name=f"ps_l3_{i}")
                for r in range(2):
                    y0 = r * 16
                    base = (2 * y0) * P2A
                    psv = ps[:, r * 512:r * 512 + 512].rearrange(
                        "q (a b) -> q a b", b=32)
                    rest = [[2 * P2A, 16], [2, 32]]
                    for kx in range(3):
                        mmdr(psv, cW3D[:, kx * 256:(kx + 1) * 256], 128,
                             a2f, base + kx, P2A, rest, kx == 0, False)
                    mmdr(psv, cW3D3[:], 128, a2f, base + 2 * P2A, S2A, rest,
                         False, False)
                    mmdr(psv, cW3S[:], 128, a2f, base + 2 * P2A + 2, S2A,
                         rest, False, True, zero_j1=True)
                dst = _ap(A3[p][:, :], P3A + 1, [[P3A, 32], [1, 32]])
                src = ps[:, 0:1024].rearrange("p (a b) -> p a b", b=32)
                nc.vector.tensor_scalar(dst, src, cT3[:], 2.0,
                                        OP.is_gt, OP.mult)
                ql = P3A + 1
                qh = 32 * P3A + 33
                nc.gpsimd.dma_start(
                    A3[p][:, S3A + ql - 1:S3A + qh - 1], A3[p][:, ql:qh])

            def l4_block(i):
                """L4: a-half (M=128) -> ACT Sign +-1; b-half (M=64) -> DVE
                {0,2}. One drain each."""
                p = i % 2
                a3f = A3[p][:, :]
                psa = pa.tile([128, 1024], F32, tag="mm", name=f"ps_l4a_{i}")
                psb = pa.tile([128, 1024], F32, tag="mm",
                              name=f"ps_l4b_{i}")[0:64, :]
                for ci in range(2):
                    q0 = (16 * ci) * P3A
                    rest = [[P3A, 16], [1, 32]]
                    for mb in range(2):
                        psx = (psa if mb == 0 else psb)[
                            :, ci * 512:ci * 512 + 512]
                        wd = cW4DA if mb == 0 else cW4DB
                        wd3 = cW4D3A if mb == 0 else cW4D3B
                        ws = cW4SA if mb == 0 else cW4SB
                        mw = 128 if mb == 0 else 64
                        for kx in range(3):
                            mmdr(psx, wd[:, kx * 2 * mw:(kx + 1) * 2 * mw],
                                 mw, a3f, q0 + kx, P3A, rest, kx == 0, False)
                        mmdr(psx, wd3[:, 0:2 * mw], mw, a3f, q0 + 2 * P3A,
                             S3A, rest, False, False)
                        mmdr(psx, ws[:, 0:2 * mw], mw, a3f,
                             q0 + 2 * P3A + 2, S3A, rest, False, True,
                             zero_j1=True)
                if i % 2 == 0:
                    nc.scalar.activation(A4[p][:, 0:NPOS], psa[:, 0:1024],
                                         ACT.Sign, bias=cNT4A[:], scale=1.0)
                else:
                    # odd images: {0,2} A4a on DVE to balance ACT vs DVE
                    nc.vector.tensor_scalar(A4[p][:, 0:NPOS], psa[:, 0:1024],
                                            cT4AD[:], 2.0, OP.is_gt, OP.mult)
                nc.vector.tensor_scalar(A4[p][0:64, NPOS:2 * NPOS],
                                        psb[:, 0:1024], cT4B[:], 2.0,
                                        OP.is_gt, OP.mult)

            def l5_block(i):
                """1x1 conv (K=192 via DR j over A4 blocks) + relu + GAP
                accum, both halves on ACT."""
                p = i % 2
                a4f = A4[p][:, :]
                psa = pa.tile([128, 1024], F32, tag="mm", name=f"ps_l5a_{i}")
                psb = pa.tile([128, 1024], F32, tag="mm",
                              name=f"ps_l5b_{i}")[0:64, :]
                for c in range(2):
                    rest = [[1, 512]]
                    mmdr(psa[:, c * 512:c * 512 + 512], cW5DA[:], 128,
                         a4f, c * 512, NPOS, rest, True, True)
                    mmdr(psb[:, c * 512:c * 512 + 512], cW5DB[:], 64,
                         a4f, c * 512, NPOS, rest, True, True)
                ba = cS1A if i % 2 == 0 else cS1AO
                bb = cS1B if i % 2 == 0 else cS1BO
                nc.scalar.activation(
                    SCRa[p][:], psa[:, 0:1024], ACT.Relu, bias=ba[:],
                    scale=1.0, accum_out=MACCa[:, i:i + 1])
                nc.scalar.activation(
                    SCRb[p][:], psb[:, 0:1024], ACT.Relu, bias=bb[:],
                    scale=1.0, accum_out=MsumB[0:64, i:i + 1])

            # ---------------- main pipeline -----------------------------------
            # reps pipeline across the boundary: pair-0 conv1 of rep r+1 is
            # interleaved into rep r's last pair block, so the marginal rep
            # cost equals steady-state throughput.
            for _rep in range(reps):
                if _rep == 0:
                    ic_load(0)
                    for r in range(8):
                        c1_chunk(0, r)
                for k in range(NPAIR):
                    a, b = 2 * k, 2 * k + 1
                    nk = k + 1
                    has_next = nk < NPAIR or _rep + 1 < reps
                    if has_next:
                        if nk == NPAIR:
                            a1_pads(1, -1.0)  # next rep's image 1 is +-1
                        ic_load(nk % NPAIR)
                    # L2 of both images, with next pair's conv1 interleaved
                    for g, img in ((0, a), (1, b)):
                        for c in range(8):
                            l2_chunk(img, c)
                            if c % 2 == 1 and has_next:
                                c1_chunk(nk % NPAIR, 4 * g + (c - 1) // 2)
                        if k == 0 and g == 1:
                            # buf 1 back to {0,2} pads for images 5/9/13
                            a1_pads(1, 0.0)
                    l3_block(a)
                    l3_block(b)
                    l4_block(a)
                    l4_block(b)
                    l5_block(a)
                    l5_block(b)

                # ---------------- GAP/FC/softmax tail -------------------------
                psf = pa.tile([128, 1024], F32, tag="mm",
                              name=f"ps_fc_{_rep}")[0:16, 0:12]
                nc.tensor.matmul(psf, MACCa[:, 0:B], cWTa[:],
                                 start=True, stop=False)
                nc.tensor.matmul(psf, MsumB[:, 0:B], cWTb[:],
                                 start=False, stop=True)

                negmax = wp.tile([16, 1], F32, tag="negmax",
                                 name=f"negmax{_rep}")
                esum = wp.tile([16, 1], F32, tag="esum", name=f"esum{_rep}")
                rsum = wp.tile([16, 1], F32, tag="rsum", name=f"rsum{_rep}")
                etile = wp.tile([16, 12], F32, tag="etile",
                                name=f"etile{_rep}")
                yout = wp.tile([16, 12], F32, tag="yout", name=f"yout{_rep}")

                nc.vector.tensor_reduce(negmax[:], psf, axis=AX.X, op=OP.max,
                                        negate=True)
                nc.scalar.activation(etile[:], psf, ACT.Exp, bias=negmax[:],
                                     scale=1.0, accum_out=esum[:])
                nc.vector.reciprocal(rsum[:], esum[:])
                nc.vector.tensor_scalar(yout[:], etile[:], rsum[:], None,
                                        OP.mult)
                nc.sync.dma_start(dY[:], yout[:])

    nc.compile()
    _CACHE[key] = nc
    return _CACHE


def _host_prep(inputs):
    """Fold BN into thresholds/affines; sign-binarize weights; build per-core
    input maps."""
    f32 = np.float32
    fp8 = mybir.dt.np(FP8)

    x = np.asarray(inputs["x"], f32)

    def inv(l):
        return (np.asarray(inputs[f"bn{l}_g"], f32)
                / np.sqrt(np.asarray(inputs[f"bn{l}_v"], f32) + np.float32(EPS)))

    invs = {l: inv(l) for l in (1, 2, 3, 4, 5)}
    for l in (1, 2, 3, 4, 5):
        assert (invs[l] > 0).all(), f"bn{l} scale not positive"

    def thr(l):
        return (np.asarray(inputs[f"bn{l}_m"], f32)
                - np.asarray(inputs[f"bn{l}_b"], f32) / invs[l])

    t1 = (thr(1) - np.asarray(inputs["conv1_b"], f32)).reshape(64, 1)
    a5 = invs[5]
    b5 = (np.asarray(inputs["bn5_b"], f32)
          - np.asarray(inputs["bn5_m"], f32) * invs[5])

    def rnd10(a):
        """Round-to-nearest-even to 10 mantissa bits (tf32-representable)."""
        i = np.ascontiguousarray(a, f32).view(np.uint32)
        keep = np.uint32(0xFFFFE000)
        lsb = (i >> np.uint32(13)) & np.uint32(1)
        out = (i + np.uint32(0x0FFF) + lsb) & keep
        return out.view(f32)

    # conv1 weights -> lhsT [tap, cout]; split-precision K=27 stack
    # [W_hi; W_hi; W_lo], replicated at partition rows 0/64
    w1 = np.asarray(inputs["conv1_w"], f32)           # [64,1,3,3]
    w1t = np.ascontiguousarray(w1[:, 0].reshape(64, 9).T)  # [9, 64]
    w1hi = rnd10(w1t)
    w1lo = rnd10(w1t - w1hi)
    w1t4 = np.zeros((128, 64), f32)
    for base in (0, 64):
        w1t4[base:base + 9] = w1hi
        w1t4[base + 9:base + 18] = w1hi
        w1t4[base + 18:base + 27] = w1lo

    sw2 = np.sign(np.asarray(inputs["w2"], f32))       # [128,64,3,3]
    sw3 = np.sign(np.asarray(inputs["w3"], f32))       # [128,128,3,3]
    sw4 = np.sign(np.asarray(inputs["w4"], f32))       # [192,128,3,3]
    sw5 = np.sign(np.asarray(inputs["w5"], f32))       # [192,192,1,1]

    # L2 DoubleRow packs: partitions = [ci(64) x ky-half], j = second tap dim
    # MM1: j -> kx in {0,1} over ky-halves {0,1}
    w2d1 = np.zeros((128, 2, 128), f32)
    for h in range(2):
        for j in range(2):
            w2d1[64 * h:64 * (h + 1), j] = sw2[:, :, h, j].T
    # MM2: j0 -> (ky=h, kx=2); j1 -> (2,0) on lo half, zero on hi half
    w2d2 = np.zeros((128, 2, 128), f32)
    for h in range(2):
        w2d2[64 * h:64 * (h + 1), 0] = sw2[:, :, h, 2].T
    w2d2[0:64, 1] = sw2[:, :, 2, 0].T
    # MM3 (lo partitions only): j0 -> (2,1); j1 -> (2,2)
    w2d3 = np.zeros((64, 2, 128), f32)
    w2d3[:, 0] = sw2[:, :, 2, 1].T
    w2d3[:, 1] = sw2[:, :, 2, 2].T

    # L3 DR packs: w3d[kx]: j=ky in {0,1}; w3d3: j=kx in {0,1} at ky=2;
    # w3s2: j0=(2,2), j1=zero
    w3d = np.zeros((128, 3, 2, 128), f32)
    for kx in range(3):
        for j in range(2):
            w3d[:, kx, j] = sw3[:, :, j, kx].T
    w3d3 = np.zeros((128, 2, 128), f32)
    for j in range(2):
        w3d3[:, j] = sw3[:, :, 2, j].T
    w3s2 = np.zeros((128, 2, 128), f32)
    w3s2[:, 0] = sw3[:, :, 2, 2].T

    # L4 DR packs: w4d[kx]: j=ky in {0,1}; w4d3: j=kx in {0,1} at ky=2;
    # w4s*2: j0=(2,2), j1=zero
    w4da = np.zeros((128, 3, 2, 128), f32)
    w4db = np.zeros((128, 3, 2, 64), f32)
    for kx in range(3):
        for j in range(2):
            w4da[:, kx, j] = sw4[:128, :, j, kx].T
            w4db[:, kx, j] = sw4[128:, :, j, kx].T
    w4d3a = np.zeros((128, 2, 128), f32)
    w4d3b = np.zeros((128, 2, 64), f32)
    for j in range(2):
        w4d3a[:, j] = sw4[:128, :, 2, j].T
        w4d3b[:, j] = sw4[128:, :, 2, j].T
    w4sa2 = np.zeros((128, 2, 128), f32)
    w4sa2[:, 0] = sw4[:128, :, 2, 2].T
    w4sb2 = np.zeros((128, 2, 64), f32)
    w4sb2[:, 0] = sw4[128:, :, 2, 2].T

    # L5 DR packs: j0 = channels 0-127; j1 = channels 128-191 (partitions
    # 0-63, zeros above)
    w5 = sw5[:, :, 0, 0]                               # [co=192, ci=192]
    w5da = np.zeros((128, 2, 128), f32)
    w5da[:, 0] = w5[:128, :128].T
    w5da[0:64, 1] = w5[:128, 128:].T
    w5db = np.zeros((128, 2, 64), f32)
    w5db[:, 0] = w5[128:, :128].T
    w5db[0:64, 1] = w5[128:, 128:].T

    # thresholds for the binarize drains (input-encoding dependent):
    # A1 {0,2} -> T2 = 2 t2 ; A2 +-1 -> T3 = 2 t3 - sum(w3) ;
    # A3 {0,2} -> T4 = 2 t4
    nt2 = (-2.0 * thr(2)).reshape(128, 1)
    sumw2 = sw2.sum(axis=(1, 2, 3))
    nt2p = (-(2.0 * thr(2) - sumw2)).reshape(128, 1)
    sumw3 = sw3.sum(axis=(1, 2, 3))
    t3 = (2.0 * thr(3) - sumw3).reshape(128, 1)
    t4 = thr(4)
    nt4a = (-2.0 * t4[:128]).reshape(128, 1)
    t4b = (2.0 * t4[128:]).reshape(64, 1)

    # L5: psum = 2*c5_01 - corr_a (A4a +-1, A4b {0,2});
    # h5 = (a5/2) * relu(psum + corr_a + 2 b5/a5)
    corr_a = w5[:, :128].sum(axis=1)                   # [192]
    s1 = corr_a + 2.0 * b5 / a5
    s1a = s1[:128].reshape(128, 1)
    s1b = s1[128:].reshape(64, 1)
    # odd images: A4a is {0,2} so the -sum(w) correction vanishes
    s1o = 2.0 * b5 / a5
    s1ao = s1o[:128].reshape(128, 1)
    s1bo = s1o[128:].reshape(64, 1)

    fc_w = np.asarray(inputs["fc_w"], f32)
    c6w = np.asarray(inputs["conv6_w"], f32)[:, :, 0, 0]   # [12, 192]
    Wp = (fc_w @ c6w) / np.float32(NPOS)               # [12, 192]
    Wp = Wp * (a5 / 2.0)[None, :]
    cvec = fc_w @ np.asarray(inputs["conv6_b"], f32) + np.asarray(
        inputs["fc_b"], f32)                           # [12]
    wta = np.ascontiguousarray(Wp[:, :128].T)          # [128, 12]
    wtb = np.zeros((65, 12), f32)
    wtb[:64] = Wp[:, 128:].T
    wtb[64] = cvec

    shared = {
        "w1t4": w1t4.astype(f32),
        "w2d1": w2d1.reshape(128, 256).astype(fp8),
        "w2d2": w2d2.reshape(128, 256).astype(fp8),
        "w2d3": w2d3.reshape(64, 256).astype(fp8),
        "w3d": w3d.reshape(128, 768).astype(fp8),
        "w3d3": w3d3.reshape(128, 256).astype(fp8),
        "w3s2": w3s2.reshape(128, 256).astype(fp8),
        "w4da": w4da.reshape(128, 768).astype(fp8),
        "w4d3a": w4d3a.reshape(128, 256).astype(fp8),
        "w4sa2": w4sa2.reshape(128, 256).astype(fp8),
        "w4db": w4db.reshape(128, 384).astype(fp8),
        "w4d3b": w4d3b.reshape(128, 128).astype(fp8),
        "w4sb2": w4sb2.reshape(128, 128).astype(fp8),
        "w5da": w5da.reshape(128, 256).astype(fp8),
        "w5db": w5db.reshape(128, 128).astype(fp8),
        "t1": t1.astype(f32), "nt1": (-t1).astype(f32),
        "nt2": nt2.astype(f32), "nt2p": nt2p.astype(f32),
        "t3": t3.astype(f32),
        "nt4a": nt4a.astype(f32),
        "t4ad": (-nt4a).astype(f32), "t4b": t4b.astype(f32),
        "s1a": s1a.astype(f32), "s1b": s1b.astype(f32),
        "s1ao": s1ao.astype(f32), "s1bo": s1bo.astype(f32),
        "wta": wta.astype(f32), "wtb": wtb.astype(f32),
    }
    # host im2col: cols[b, 3*ky+kx, y*64+x] = xpad[b, 2y+ky, 2x+kx];
    # split-precision blocks [X_hi; X_lo; X_hi] (see dX comment in _build)
    xpad = np.pad(x[:, 0], ((0, 0), (1, 1), (1, 1)))
    cols = np.stack([xpad[:, ky:ky + 127:2, kx:kx + 127:2]
                     for ky in range(3) for kx in range(3)],
                    axis=1).reshape(x.shape[0], 9, 4096)
    chi = rnd10(cols)
    clo = rnd10(cols - chi)
    cols27 = np.concatenate([chi, clo, chi], axis=1)   # [B*8, 27, 4096]
    in_maps = []
    for c in range(N_CORES):
        m = dict(shared)
        m["x"] = np.ascontiguousarray(cols27[c * B:(c + 1) * B])
        in_maps.append(m)
    return in_maps


def kernel(**inputs):
    cache = _build()
    in_maps = _host_prep(inputs)
    res = run_bass_kernel_spmd(cache["nc1"], in_maps,
                               core_ids=list(range(N_CORES)))
    _CACHE["last_results"] = res
    return np.concatenate([res.results[c]["y"] for c in range(N_CORES)],
                          axis=0)


# revision 35
# speedup vs baseline: 4.4483x; 4.4483x over previous
"""Trainium2 Bass kernel for BinarizedInputNetwork (v2).

Contract: kernel(**inputs) takes the FULL unsharded inputs (batch 128) and
returns the FULL [128, 12] float32 softmax output. Internally shards the
batch across 8 NeuronCores (16 images each), runs one SPMD Bass program.

Network (per image, input [1,128,128]):
  conv1 3x3 s2 p1 (1->64)  + BN + ReLU -> sign       => binary acts
  conv2 3x3 s1 p1 (64->128, sign wts)  + BN + ReLU -> sign
  conv3 3x3 s2 p1 (128->128, sign wts) + BN + ReLU -> sign
  conv4 3x3 s1 p1 (128->192, sign wts) + BN + ReLU -> sign
  conv5 1x1 s1 p0 (192->192, sign wts) + BN + ReLU
  conv6 1x1 (192->12) + b ; GAP ; FC 12x12 + b ; softmax

Device mapping (changes vs v1):
  - conv1 runs in fp32r (full-rate on PE vs 4x-slow fp32; ~2e-4 rel err,
    harmless before binarization).
  - Binary activations use TWO exact encodings so the psum->sbuf binarize
    drains can be split across BOTH VectorE and ScalarE (v1 was DVE-bound):
      {0,2} = 2*a   via DVE tensor_scalar (is_gt t)*2
      {-1,1} = 2a-1 via ScalarE activation Sign(x - t)  (pads = -1)
    Thresholds/affines absorb the encoding exactly (integer arithmetic in
    fp8 stays exact): for +-1 inputs the next layer's threshold gains a
    -sum(w) correction; for {0,2} inputs thresholds just double.
    Assignment: A1 {0,2} (DVE), A2 +-1 (ACT), A3 {0,2} (DVE),
    A4a +-1 (ACT), A4b {0,2} (DVE), L5 relu+GAP-accum on ACT.
  - PSUM tiles span 2 banks ([*,1024]) so each drain instruction covers
    1024 columns (half the per-instruction overhead of v1's 512).
  - Every fp8 matmul uses DoubleRow; former solo taps get a zero-weighted
    second j slot (0.5 vs 1.0 PE cycles per output column).
  - conv1 matmuls for the NEXT image pair are interleaved into the PE
    stream between L2 chunks so slow c1 drains never head-block the PE.
"""

import sys

sys.path.insert(0, "/opt/trn_rl_repo")

import numpy as np

import concourse.ap as apm
import concourse.bass as bass
import concourse.mybir as mybir
import concourse.bacc as bacc
import concourse.tile as tile
from concourse.bass_utils import run_bass_kernel_spmd

F32 = mybir.dt.float32
F32R = mybir.dt.float32r
FP8 = mybir.dt.float8e4
AX = mybir.AxisListType
OP = mybir.AluOpType
ACT = mybir.ActivationFunctionType
DR = mybir.MatmulPerfMode.DoubleRow

N_CORES = 8
B = 16  # images per core
NPAIR = B // 2

EPS = 1e-5

# geometry
H1, W1 = 64, 64          # conv1 output spatial
P1 = W1 + 2              # padded width/height for A1 (66)
S1 = P1 * P1             # 4356
BO1 = 4368               # A1 shifted-copy block offset (16-aligned)
H3, W3 = 32, 32          # conv3 output spatial
P3 = W3 + 2              # 34
P2A = 80                 # A2 row pitch (66 rows x 80)
S2A = 66 * P2A           # 5280
P3A = 48                 # A3 row pitch (34 rows x 48)
S3A = P3 * P3A           # 1632
NPOS = H3 * W3           # 1024 valid positions for L5/GAP

_CACHE = {}


def _ap(base2d, off, dims):
    """Custom AP over an SBUF tile slice: base partition dim + free dims
    (supports overlapping patterns rearrange can't express)."""
    return apm.AP(tensor=base2d.tensor, offset=base2d.offset + off,
                  ap=[list(base2d.ap[0])] + [list(d) for d in dims])


def _build(reps=1, sim_compat=False):
    """Trace + compile the Bass program (cached). reps>1 replicates the whole
    pipeline on-device (for timing via wall-clock differencing).

    sim_compat=True lowers every DoubleRow matmul to two plain accumulating
    matmuls (identical math + AP offsets) because the exec interpreter only
    supports 3-dim DR rhs APs. Used by sim_test.py only."""
    key = f"nc{reps}" + ("s" if sim_compat else "")
    if key in _CACHE:
        return _CACHE

    nc = bacc.Bacc("TRN2", target_bir_lowering=False, debug=False,
                   num_devices=N_CORES)

    # ---- DRAM I/O ----
    # host-side im2col of the 1-channel input, split-precision for exact
    # fp32r matmul: rows 0-8 = X_hi (tf32-rounded), 9-17 = X_lo (residual,
    # tf32-rounded), 18-26 = X_hi again. Paired with lhsT rows
    # [W_hi; W_hi; W_lo] this computes W@X to ~2^-20 despite fp32r inputs,
    # at the same 1-cycle/row PE rate (cost is K-independent).
    dX = nc.dram_tensor("x", [B, 27, 4096], F32R, kind="ExternalInput").ap()
    # conv1 weights at partition rows 0-8 and 64-72 (2-way row tiling)
    dW1T = nc.dram_tensor("w1t4", [128, 64], F32R, kind="ExternalInput").ap()
    dW2D1 = nc.dram_tensor("w2d1", [128, 256], FP8, kind="ExternalInput").ap()
    dW2D2 = nc.dram_tensor("w2d2", [128, 256], FP8, kind="ExternalInput").ap()
    dW2D3 = nc.dram_tensor("w2d3", [64, 256], FP8, kind="ExternalInput").ap()
    dW3D = nc.dram_tensor("w3d", [128, 768], FP8, kind="ExternalInput").ap()
    dW3D3 = nc.dram_tensor("w3d3", [128, 256], FP8, kind="ExternalInput").ap()
    dW3S = nc.dram_tensor("w3s2", [128, 256], FP8, kind="ExternalInput").ap()
    dW4DA = nc.dram_tensor("w4da", [128, 768], FP8, kind="ExternalInput").ap()
    dW4D3A = nc.dram_tensor("w4d3a", [128, 256], FP8, kind="ExternalInput").ap()
    dW4SA = nc.dram_tensor("w4sa2", [128, 256], FP8, kind="ExternalInput").ap()
    dW4DB = nc.dram_tensor("w4db", [128, 384], FP8, kind="ExternalInput").ap()
    dW4D3B = nc.dram_tensor("w4d3b", [128, 128], FP8, kind="ExternalInput").ap()
    dW4SB = nc.dram_tensor("w4sb2", [128, 128], FP8, kind="ExternalInput").ap()
    dW5DA = nc.dram_tensor("w5da", [128, 256], FP8, kind="ExternalInput").ap()
    dW5DB = nc.dram_tensor("w5db", [128, 128], FP8, kind="ExternalInput").ap()
    dT1 = nc.dram_tensor("t1", [64, 1], F32, kind="ExternalInput").ap()
    dNT1 = nc.dram_tensor("nt1", [64, 1], F32, kind="ExternalInput").ap()
    dNT2 = nc.dram_tensor("nt2", [128, 1], F32, kind="ExternalInput").ap()
    dNT2P = nc.dram_tensor("nt2p", [128, 1], F32, kind="ExternalInput").ap()
    dT3 = nc.dram_tensor("t3", [128, 1], F32, kind="ExternalInput").ap()
    dNT4A = nc.dram_tensor("nt4a", [128, 1], F32, kind="ExternalInput").ap()
    dT4AD = nc.dram_tensor("t4ad", [128, 1], F32, kind="ExternalInput").ap()
    dT4B = nc.dram_tensor("t4b", [64, 1], F32, kind="ExternalInput").ap()
    dS1A = nc.dram_tensor("s1a", [128, 1], F32, kind="ExternalInput").ap()
    dS1B = nc.dram_tensor("s1b", [64, 1], F32, kind="ExternalInput").ap()
    dS1AO = nc.dram_tensor("s1ao", [128, 1], F32, kind="ExternalInput").ap()
    dS1BO = nc.dram_tensor("s1bo", [64, 1], F32, kind="ExternalInput").ap()
    dWTa = nc.dram_tensor("wta", [128, 12], F32, kind="ExternalInput").ap()
    dWTb = nc.dram_tensor("wtb", [65, 12], F32, kind="ExternalInput").ap()
    dY = nc.dram_tensor("y", [B, 12], F32, kind="ExternalOutput").ap()

    with tile.TileContext(nc) as tc:
        with tc.tile_pool(name="const", bufs=1) as cp, \
             tc.tile_pool(name="work", bufs=2) as wp, \
             tc.tile_pool(name="psA", bufs=2, space="PSUM") as pa, \
             tc.tile_pool(name="psC", bufs=2, space="PSUM") as pc:

            def ctile(name, shape, dtype):
                return cp.tile(shape, dtype, tag=name, name=name)

            # ---- persistent weight/param tiles ----
            cW1T = ctile("cW1T", [128, 64], F32R)
            cW2D1 = ctile("cW2D1", [128, 256], FP8)
            cW2D2 = ctile("cW2D2", [128, 256], FP8)
            cW2D3 = ctile("cW2D3", [64, 256], FP8)
            cW3D = ctile("cW3D", [128, 768], FP8)
            cW3D3 = ctile("cW3D3", [128, 256], FP8)
            cW3S = ctile("cW3S", [128, 256], FP8)
            cW4DA = ctile("cW4DA", [128, 768], FP8)
            cW4D3A = ctile("cW4D3A", [128, 256], FP8)
            cW4SA = ctile("cW4SA", [128, 256], FP8)
            cW4DB = ctile("cW4DB", [128, 384], FP8)
            cW4D3B = ctile("cW4D3B", [128, 128], FP8)
            cW4SB = ctile("cW4SB", [128, 128], FP8)
            cW5DA = ctile("cW5DA", [128, 256], FP8)
            cW5DB = ctile("cW5DB", [128, 128], FP8)
            cT1 = ctile("cT1", [64, 1], F32)
            cNT1 = ctile("cNT1", [64, 1], F32)
            cNT2 = ctile("cNT2", [128, 1], F32)
            cNT2P = ctile("cNT2P", [128, 1], F32)
            cT3 = ctile("cT3", [128, 1], F32)
            cNT4A = ctile("cNT4A", [128, 1], F32)
            cT4AD = ctile("cT4AD", [128, 1], F32)
            cT4B = ctile("cT4B", [64, 1], F32)
            cS1A = ctile("cS1A", [128, 1], F32)
            cS1B = ctile("cS1B", [64, 1], F32)
            cS1AO = ctile("cS1AO", [128, 1], F32)
            cS1BO = ctile("cS1BO", [64, 1], F32)
            cWTa = ctile("cWTa", [128, 12], F32)
            cWTb = ctile("cWTb", [65, 12], F32)

            # load-order matters at startup: conv1's deps go first on the
            # HWDGE (sync) queue so pair-0 conv1 starts right after the IC
            # DMA; L2 weights ride the Pool SWDGE path in parallel; the rest
            # (first needed several us in) go via the scalar engine's queue.
            for t_, d_ in [(cW1T, dW1T), (cT1, dT1), (cNT1, dNT1),
                           (cNT2, dNT2), (cNT2P, dNT2P)]:
                nc.sync.dma_start(t_[:], d_[:])
            for t_, d_ in [(cW2D1, dW2D1), (cW2D2, dW2D2), (cW2D3, dW2D3)]:
                nc.gpsimd.dma_start(t_[:], d_[:])
            for t_, d_ in [(cW3D, dW3D), (cW3D3, dW3D3), (cW3S, dW3S),
                           (cW4DA, dW4DA), (cW4D3A, dW4D3A), (cW4SA, dW4SA),
                           (cW4DB, dW4DB), (cW4D3B, dW4D3B),
                           (cW4SB, dW4SB), (cW5DA, dW5DA),
                           (cW5DB, dW5DB),
                           (cT3, dT3), (cNT4A, dNT4A), (cT4AD, dT4AD),
                           (cT4B, dT4B),
                           (cS1A, dS1A), (cS1B, dS1B),
                           (cS1AO, dS1AO), (cS1BO, dS1BO),
                           (cWTa, dWTa), (cWTb, dWTb)]:
                nc.scalar.dma_start(t_[:], d_[:])

            # ---- persistent activation buffers ----
            # IC: image pair (g=0 at partitions 0-8, g=1 at 64-72)
            IC = [ctile(f"IC{p}", [128, 4096], F32R) for p in range(2)]
            # A1 {0,2}: block0 [lo=img, hi=img row+1]; blk1 = +1; blk2 = +130
            A1 = [ctile(f"A1_{p}", [128, 3 * BO1], FP8) for p in range(4)]
            # A2 +-1 (pad -1), pitch 80; +1-col copy block at S2A
            A2 = [ctile(f"A2_{p}", [128, 2 * S2A], FP8) for p in range(2)]
            # A3 {0,2} (pad 0), pitch 48; +1-col copy block at S3A
            A3 = [ctile(f"A3_{p}", [128, 2 * S3A], FP8) for p in range(2)]
            # A4: block0 = couts 0-127 (+-1); block1 at NPOS = couts 128-191
            # {0,2} on partitions 0-63, zeros above
            A4 = [ctile(f"A4_{p}", [128, 2 * NPOS], FP8) for p in range(2)]
            MACCa = ctile("MACCa", [128, B], F32)
            MsumB = ctile("MsumB", [65, B], F32)
            SCRa = [ctile(f"SCRa{p}", [128, 1024], F32) for p in range(2)]
            SCRb = [ctile(f"SCRb{p}", [64, 1024], F32) for p in range(2)]

            # ---- one-time pad/guard memsets (pad-only, cheap) ----
            def a1_pads(p, val):
                """Pad regions of A1[p] that encode 'activation 0': 0.0 for
                {0,2} images, -1.0 for +-1 images (pair-0 image 1 only)."""
                a1 = A1[p]
                # row 0 + row 65 + tail of block0
                nc.gpsimd.memset(a1[:, 0:P1], val)
                nc.gpsimd.memset(a1[:, 65 * P1:BO1], val)
                # cols 65,0 pairs of rows 0..64: [[66,65],[1,2]] at offset 65
                nc.gpsimd.memset(_ap(a1[:, :], 65, [[P1, 65], [1, 2]]), val)

            for p in range(4):
                a1 = A1[p]
                a1_pads(p, -1.0 if p == 1 else 0.0)
                # blk copy tails never covered by the per-image copies;
                # blk2 hi half is only read zero-weighted, so it is copied
                # lo-only and memset here
                nc.gpsimd.memset(a1[:, BO1 + 4355:2 * BO1], 0.0)
                nc.gpsimd.memset(a1[0:64, 2 * BO1 + 4226:3 * BO1], 0.0)
                nc.gpsimd.memset(a1[64:128, 2 * BO1:3 * BO1], 0.0)
            for p in range(2):
                a2 = A2[p]
                nc.gpsimd.memset(a2[:, 0:P2A], -1.0)
                nc.gpsimd.memset(a2[:, 65 * P2A:S2A], -1.0)
                nc.gpsimd.memset(
                    _ap(a2[:, :], 65, [[P2A, 65], [1, 16]]), -1.0)
                # blk1: rows 0 and 65 + col gaps (copies only cover interior)
                nc.gpsimd.memset(a2[:, S2A:S2A + P2A], -1.0)
                nc.gpsimd.memset(a2[:, S2A + 65 * P2A:2 * S2A], -1.0)
                nc.gpsimd.memset(
                    _ap(a2[:, :], S2A + 64, [[P2A, 65], [1, 17]]), -1.0)
                a3 = A3[p]
                nc.gpsimd.memset(a3[:, 0:P3A], 0.0)
                nc.gpsimd.memset(a3[:, 33 * P3A:S3A], 0.0)
                nc.gpsimd.memset(
                    _ap(a3[:, :], 33, [[P3A, 33], [1, 16]]), 0.0)
                nc.gpsimd.memset(a3[:, S3A:S3A + P3A], 0.0)
                nc.gpsimd.memset(a3[:, S3A + 33 * P3A:2 * S3A], 0.0)
                nc.gpsimd.memset(
                    _ap(a3[:, :], S3A + 32, [[P3A, 33], [1, 17]]), 0.0)
                nc.gpsimd.memset(A4[p][64:128, NPOS:2 * NPOS], 0.0)
            nc.vector.memset(MsumB[64:65, :], 1.0)

            def mmdr(out, wflat, m, rhs_tile, off0, jstep, rest, start, stop,
                     zero_j1=False):
                """DoubleRow matmul: lhsT = wflat viewed [p, 2, m], rhs j-pair
                at (off0, off0+jstep). zero_j1: second slot is zero weights
                (rhs garbage, masked). sim_compat lowers to 2 plain matmuls."""
                if not sim_compat:
                    nc.tensor.matmul(
                        out, wflat.rearrange("p (j m) -> p j m", j=2),
                        _ap(rhs_tile, off0, [[jstep, 2]] + rest),
                        start=start, stop=stop, perf_mode=DR)
                    return
                nc.tensor.matmul(out, wflat[:, 0:m],
                                 _ap(rhs_tile, off0, rest),
                                 start=start, stop=stop and zero_j1)
                if not zero_j1:
                    nc.tensor.matmul(out, wflat[:, m:2 * m],
                                     _ap(rhs_tile, off0 + jstep, rest),
                                     start=False, stop=stop)

            # ---------------- layer bodies ------------------------------------
            def ic_load(k):
                """DMA im2col input for pair k."""
                pq = k % 2
                nc.sync.dma_start(IC[pq][0:27, :], dX[2 * k])
                nc.sync.dma_start(IC[pq][64:91, :], dX[2 * k + 1])

            c1_state = {}

            def c1_chunk(k, r):
                """conv1 r-chunk (8 out rows) for both images of pair k.
                Tiles [64,1024] hold 2 r-chunks; drain (DVE, {0,2}) + A1
                copies fire after the odd r."""
                pq = k % 2
                for g in range(2):
                    if r % 2 == 0:
                        c1_state[g] = pc.tile([64, 1024], F32, tag="c1",
                                              name=f"ps_c1_{k}_{r}_{g}")
                    ps = c1_state[g]
                    nc.tensor.matmul(
                        ps[:, (r % 2) * 512:(r % 2) * 512 + 512],
                        cW1T[64 * g:64 * g + 27, :],
                        IC[pq][64 * g:64 * g + 27, r * 512:(r + 1) * 512],
                        start=True, stop=True,
                        tile_position=(64 * g, 0))
                if r % 2 == 1:
                    y0 = 8 * (r - 1)
                    for g in range(2):
                        a1t = A1[(2 * k + g) % 4]
                        dst = _ap(a1t[0:64, :], (y0 + 1) * P1 + 1,
                                  [[P1, 16], [1, 64]])
                        src = c1_state[g][:, 0:1024].rearrange(
                            "p (a b) -> p a b", b=64)
                        if k == 0 and g == 1:
                            # pair 0 has no other work to hide its drains:
                            # split them across engines (image 1 -> ACT, +-1)
                            nc.scalar.activation(dst, src, ACT.Sign,
                                                 bias=cNT1[:], scale=1.0)
                        else:
                            nc.vector.tensor_scalar(dst, src, cT1[:], 2.0,
                                                    OP.is_gt, OP.mult)
                if r == 7:
                    for g in range(2):
                        a1t = A1[(2 * k + g) % 4]
                        # hi half: copy2[q] = copy1[q + P1]. Issued from the
                        # Pool engine (SWDGE) to keep the SP sequencer and
                        # shared HWDGE off the critical path.
                        nc.gpsimd.dma_start(a1t[64:128, 0:S1 - P1],
                                            a1t[0:64, P1:S1])
                        # shifted blocks for DoubleRow pair dim (blk2 is only
                        # read zero-weighted on the hi half: copy lo only)
                        nc.gpsimd.dma_start(a1t[:, BO1:BO1 + 4355],
                                            a1t[:, 1:4356])
                        nc.gpsimd.dma_start(a1t[0:64, 2 * BO1:2 * BO1 + 4226],
                                            a1t[0:64, 130:4356])

            l2_state = {}

            def l2_chunk(i, c):
                """L2 chunk c (8 out rows, valid cols only) for image i.
                Drain (ACT Sign, +-1) after odd c."""
                p = i % 2
                a1f = A1[i % 4][:, :]
                a1lo = A1[i % 4][0:64, :]
                y0 = 8 * c
                q0 = y0 * P1
                if c % 2 == 0:
                    l2_state[0] = pa.tile([128, 1024], F32, tag="mm",
                                          name=f"ps_l2_{i}_{c}")
                ps = l2_state[0]
                out = ps[:, (c % 2) * 512:(c % 2) * 512 + 512]
                rest = [[P1, 8], [1, 64]]
                mmdr(out, cW2D1[:], 128, a1f, q0, BO1, rest, True, False)
                mmdr(out, cW2D2[:], 128, a1f, q0 + 2, 2 * BO1, rest,
                     False, False)
                mmdr(out, cW2D3[:], 128, a1lo, q0 + 133, BO1, rest,
                     False, True)
                if c % 2 == 1:
                    yb = 8 * (c - 1)
                    dst = _ap(A2[p][:, :], (yb + 1) * P2A + 1,
                              [[P2A, 16], [1, 64]])
                    src = ps[:, 0:1024].rearrange("p (a b) -> p a b", b=64)
                    bias = cNT2P if i == 1 else cNT2  # image 1's A1 is +-1
                    nc.scalar.activation(dst, src, ACT.Sign, bias=bias[:],
                                         scale=1.0)
                    # +1-col shifted block for this row range
                    ql = (yb + 1) * P2A + 1
                    qh = (yb + 16) * P2A + 65
                    nc.sync.dma_start(
                        A2[p][:, S2A + ql - 1:S2A + qh - 1], A2[p][:, ql:qh])

            def l3_block(i):
                """L3 (stride 2): 2 matmul groups -> one [128,1024] DVE drain
                ({0,2})."""
                p = i % 2
                a2f = A2[p][:, :]
                ps = pa.tile([128, 1024], F32, tag="mm", name=f"ps_l3_{i}")
                for r in range(2):
                    y0 = r * 16
                    base = (2 * y0) * P2A
                    psv = ps[:, r * 512:r * 512 + 512].rearrange(
                        "q (a b) -> q a b", b=32)
                    rest = [[2 * P2A, 16], [2, 32]]
                    for kx in range(3):
                        mmdr(psv, cW3D[:, kx * 256:(kx + 1) * 256], 128,
                             a2f, base + kx, P2A, rest, kx == 0, False)
                    mmdr(psv, cW3D3[:], 128, a2f, base + 2 * P2A, S2A, rest,
                         False, False)
                    mmdr(psv, cW3S[:], 128, a2f, base + 2 * P2A + 2, S2A,
                         rest, False, True, zero_j1=True)
                dst = _ap(A3[p][:, :], P3A + 1, [[P3A, 32], [1, 32]])
                src = ps[:, 0:1024].rearrange("p (a b) -> p a b", b=32)
                nc.vector.tensor_scalar(dst, src, cT3[:], 2.0,
                                        OP.is_gt, OP.mult)
                # split the +1-col block copy at the L4 group boundary so
                # L4's first group isn't gated on the full-image copy
                qm = 18 * P3A
                ql = P3A + 1
                qh = 32 * P3A + 33
                nc.gpsimd.dma_start(
                    A3[p][:, S3A + ql - 1:S3A + qm - 1], A3[p][:, ql:qm])
                nc.gpsimd.dma_start(
                    A3[p][:, S3A + qm - 1:S3A + qh - 1], A3[p][:, qm:qh])

            def l4_block(i):
                """L4: a-half (M=128) -> ACT Sign +-1; b-half (M=64) -> DVE
                {0,2}. One drain each."""
                p = i % 2
                a3f = A3[p][:, :]
                psa = pa.tile([128, 1024], F32, tag="mm", name=f"ps_l4a_{i}")
                psb = pa.tile([128, 1024], F32, tag="mm",
                              name=f"ps_l4b_{i}")[0:64, :]
                for ci in range(2):
                    q0 = (16 * ci) * P3A
                    rest = [[P3A, 16], [1, 32]]
                    for mb in range(2):
                        psx = (psa if mb == 0 else psb)[
                            :, ci * 512:ci * 512 + 512]
                        wd = cW4DA if mb == 0 else cW4DB
                        wd3 = cW4D3A if mb == 0 else cW4D3B
                        ws = cW4SA if mb == 0 else cW4SB
                        mw = 128 if mb == 0 else 64
                        for kx in range(3):
                            mmdr(psx, wd[:, kx * 2 * mw:(kx + 1) * 2 * mw],
                                 mw, a3f, q0 + kx, P3A, rest, kx == 0, False)
                        mmdr(psx, wd3[:, 0:2 * mw], mw, a3f, q0 + 2 * P3A,
                             S3A, rest, False, False)
                        mmdr(psx, ws[:, 0:2 * mw], mw, a3f,
                             q0 + 2 * P3A + 2, S3A, rest, False, True,
                             zero_j1=True)
                if i % 2 == 0:
                    nc.scalar.activation(A4[p][:, 0:NPOS], psa[:, 0:1024],
                                         ACT.Sign, bias=cNT4A[:], scale=1.0)
                else:
                    # odd images: {0,2} A4a on DVE to balance ACT vs DVE
                    nc.vector.tensor_scalar(A4[p][:, 0:NPOS], psa[:, 0:1024],
                                            cT4AD[:], 2.0, OP.is_gt, OP.mult)
                nc.vector.tensor_scalar(A4[p][0:64, NPOS:2 * NPOS],
                                        psb[:, 0:1024], cT4B[:], 2.0,
                                        OP.is_gt, OP.mult)

            def l5_block(i):
                """1x1 conv (K=192 via DR j over A4 blocks) + relu + GAP
                accum, both halves on ACT."""
                p = i % 2
                a4f = A4[p][:, :]
                psa = pa.tile([128, 1024], F32, tag="mm", name=f"ps_l5a_{i}")
                psb = pa.tile([128, 1024], F32, tag="mm",
                              name=f"ps_l5b_{i}")[0:64, :]
                for c in range(2):
                    rest = [[1, 512]]
                    mmdr(psa[:, c * 512:c * 512 + 512], cW5DA[:], 128,
                         a4f, c * 512, NPOS, rest, True, True)
                    mmdr(psb[:, c * 512:c * 512 + 512], cW5DB[:], 64,
                         a4f, c * 512, NPOS, rest, True, True)
                ba = cS1A if i % 2 == 0 else cS1AO
                bb = cS1B if i % 2 == 0 else cS1BO
                nc.scalar.activation(
                    SCRa[p][:], psa[:, 0:1024], ACT.Relu, bias=ba[:],
                    scale=1.0, accum_out=MACCa[:, i:i + 1])
                nc.scalar.activation(
                    SCRb[p][:], psb[:, 0:1024], ACT.Relu, bias=bb[:],
                    scale=1.0, accum_out=MsumB[0:64, i:i + 1])

            # ---------------- main pipeline -----------------------------------
            # reps pipeline across the boundary: pair-0 conv1 of rep r+1 is
            # interleaved into rep r's last pair block, so the marginal rep
            # cost equals steady-state throughput.
            for _rep in range(reps):
                if _rep == 0:
                    ic_load(0)
                    for r in range(8):
                        c1_chunk(0, r)
                for k in range(NPAIR):
                    a, b = 2 * k, 2 * k + 1
                    nk = k + 1
                    has_next = nk < NPAIR or _rep + 1 < reps
                    if has_next:
                        if nk == NPAIR:
                            a1_pads(1, -1.0)  # next rep's image 1 is +-1
                        ic_load(nk % NPAIR)
                    # L2 of both images, with next pair's conv1 interleaved
                    for g, img in ((0, a), (1, b)):
                        for c in range(8):
                            l2_chunk(img, c)
                            if c % 2 == 1 and has_next:
                                c1_chunk(nk % NPAIR, 4 * g + (c - 1) // 2)
                        if k == 0 and g == 1:
                            # buf 1 back to {0,2} pads for images 5/9/13
                            a1_pads(1, 0.0)
                    l3_block(a)
                    l3_block(b)
                    l4_block(a)
                    l4_block(b)
                    l5_block(a)
                    l5_block(b)

                # ---------------- GAP/FC/softmax tail -------------------------
                psf = pa.tile([128, 1024], F32, tag="mm",
                              name=f"ps_fc_{_rep}")[0:16, 0:12]
                nc.tensor.matmul(psf, MACCa[:, 0:B], cWTa[:],
                                 start=True, stop=False)
                nc.tensor.matmul(psf, MsumB[:, 0:B], cWTb[:],
                                 start=False, stop=True)

                negmax = wp.tile([16, 1], F32, tag="negmax",
                                 name=f"negmax{_rep}")
                esum = wp.tile([16, 1], F32, tag="esum", name=f"esum{_rep}")
                rsum = wp.tile([16, 1], F32, tag="rsum", name=f"rsum{_rep}")
                etile = wp.tile([16, 12], F32, tag="etile",
                                name=f"etile{_rep}")
                yout = wp.tile([16, 12], F32, tag="yout", name=f"yout{_rep}")

                nc.vector.tensor_reduce(negmax[:], psf, axis=AX.X, op=OP.max,
                                        negate=True)
                nc.scalar.activation(etile[:], psf, ACT.Exp, bias=negmax[:],
                                     scale=1.0, accum_out=esum[:])
                nc.vector.reciprocal(rsum[:], esum[:])
                nc.vector.tensor_scalar(yout[:], etile[:], rsum[:], None,
                                        OP.mult)
                nc.sync.dma_start(dY[:], yout[:])

    nc.compile()
    _CACHE[key] = nc
    return _CACHE


def _host_prep(inputs):
    """Fold BN into thresholds/affines; sign-binarize weights; build per-core
    input maps."""
    f32 = np.float32
    fp8 = mybir.dt.np(FP8)

    x = np.asarray(inputs["x"], f32)

    def inv(l):
        return (np.asarray(inputs[f"bn{l}_g"], f32)
                / np.sqrt(np.asarray(inputs[f"bn{l}_v"], f32) + np.float32(EPS)))

    invs = {l: inv(l) for l in (1, 2, 3, 4, 5)}
    for l in (1, 2, 3, 4, 5):
        assert (invs[l] > 0).all(), f"bn{l} scale not positive"

    def thr(l):
        return (np.asarray(inputs[f"bn{l}_m"], f32)
                - np.asarray(inputs[f"bn{l}_b"], f32) / invs[l])

    t1 = (thr(1) - np.asarray(inputs["conv1_b"], f32)).reshape(64, 1)
    a5 = invs[5]
    b5 = (np.asarray(inputs["bn5_b"], f32)
          - np.asarray(inputs["bn5_m"], f32) * invs[5])

    def rnd10(a):
        """Round-to-nearest-even to 10 mantissa bits (tf32-representable)."""
        i = np.ascontiguousarray(a, f32).view(np.uint32)
        keep = np.uint32(0xFFFFE000)
        lsb = (i >> np.uint32(13)) & np.uint32(1)
        out = (i + np.uint32(0x0FFF) + lsb) & keep
        return out.view(f32)

    # conv1 weights -> lhsT [tap, cout]; split-precision K=27 stack
    # [W_hi; W_hi; W_lo], replicated at partition rows 0/64
    w1 = np.asarray(inputs["conv1_w"], f32)           # [64,1,3,3]
    w1t = np.ascontiguousarray(w1[:, 0].reshape(64, 9).T)  # [9, 64]
    w1hi = rnd10(w1t)
    w1lo = rnd10(w1t - w1hi)
    w1t4 = np.zeros((128, 64), f32)
    for base in (0, 64):
        w1t4[base:base + 9] = w1hi
        w1t4[base + 9:base + 18] = w1hi
        w1t4[base + 18:base + 27] = w1lo

    sw2 = np.sign(np.asarray(inputs["w2"], f32))       # [128,64,3,3]
    sw3 = np.sign(np.asarray(inputs["w3"], f32))       # [128,128,3,3]
    sw4 = np.sign(np.asarray(inputs["w4"], f32))       # [192,128,3,3]
    sw5 = np.sign(np.asarray(inputs["w5"], f32))       # [192,192,1,1]

    # L2 DoubleRow packs: partitions = [ci(64) x ky-half], j = second tap dim
    # MM1: j -> kx in {0,1} over ky-halves {0,1}
    w2d1 = np.zeros((128, 2, 128), f32)
    for h in range(2):
        for j in range(2):
            w2d1[64 * h:64 * (h + 1), j] = sw2[:, :, h, j].T
    # MM2: j0 -> (ky=h, kx=2); j1 -> (2,0) on lo half, zero on hi half
    w2d2 = np.zeros((128, 2, 128), f32)
    for h in range(2):
        w2d2[64 * h:64 * (h + 1), 0] = sw2[:, :, h, 2].T
    w2d2[0:64, 1] = sw2[:, :, 2, 0].T
    # MM3 (lo partitions only): j0 -> (2,1); j1 -> (2,2)
    w2d3 = np.zeros((64, 2, 128), f32)
    w2d3[:, 0] = sw2[:, :, 2, 1].T
    w2d3[:, 1] = sw2[:, :, 2, 2].T

    # L3 DR packs: w3d[kx]: j=ky in {0,1}; w3d3: j=kx in {0,1} at ky=2;
    # w3s2: j0=(2,2), j1=zero
    w3d = np.zeros((128, 3, 2, 128), f32)
    for kx in range(3):
        for j in range(2):
            w3d[:, kx, j] = sw3[:, :, j, kx].T
    w3d3 = np.zeros((128, 2, 128), f32)
    for j in range(2):
        w3d3[:, j] = sw3[:, :, 2, j].T
    w3s2 = np.zeros((128, 2, 128), f32)
    w3s2[:, 0] = sw3[:, :, 2, 2].T

    # L4 DR packs: w4d[kx]: j=ky in {0,1}; w4d3: j=kx in {0,1} at ky=2;
    # w4s*2: j0=(2,2), j1=zero
    w4da = np.zeros((128, 3, 2, 128), f32)
    w4db = np.zeros((128, 3, 2, 64), f32)
    for kx in range(3):
        for j in range(2):
            w4da[:, kx, j] = sw4[:128, :, j, kx].T
            w4db[:, kx, j] = sw4[128:, :, j, kx].T
    w4d3a = np.zeros((128, 2, 128), f32)
    w4d3b = np.zeros((128, 2, 64), f32)
    for j in range(2):
        w4d3a[:, j] = sw4[:128, :, 2, j].T
        w4d3b[:, j] = sw4[128:, :, 2, j].T
    w4sa2 = np.zeros((128, 2, 128), f32)
    w4sa2[:, 0] = sw4[:128, :, 2, 2].T
    w4sb2 = np.zeros((128, 2, 64), f32)
    w4sb2[:, 0] = sw4[128:, :, 2, 2].T

    # L5 DR packs: j0 = channels 0-127; j1 = channels 128-191 (partitions
    # 0-63, zeros above)
    w5 = sw5[:, :, 0, 0]                               # [co=192, ci=192]
    w5da = np.zeros((128, 2, 128), f32)
    w5da[:, 0] = w5[:128, :128].T
    w5da[0:64, 1] = w5[:128, 128:].T
    w5db = np.zeros((128, 2, 64), f32)
    w5db[:, 0] = w5[128:, :128].T
    w5db[0:64, 1] = w5[128:, 128:].T

    # thresholds for the binarize drains (input-encoding dependent):
    # A1 {0,2} -> T2 = 2 t2 ; A2 +-1 -> T3 = 2 t3 - sum(w3) ;
    # A3 {0,2} -> T4 = 2 t4
    nt2 = (-2.0 * thr(2)).reshape(128, 1)
    sumw2 = sw2.sum(axis=(1, 2, 3))
    nt2p = (-(2.0 * thr(2) - sumw2)).reshape(128, 1)
    sumw3 = sw3.sum(axis=(1, 2, 3))
    t3 = (2.0 * thr(3) - sumw3).reshape(128, 1)
    t4 = thr(4)
    nt4a = (-2.0 * t4[:128]).reshape(128, 1)
    t4b = (2.0 * t4[128:]).reshape(64, 1)

    # L5: psum = 2*c5_01 - corr_a (A4a +-1, A4b {0,2});
    # h5 = (a5/2) * relu(psum + corr_a + 2 b5/a5)
    corr_a = w5[:, :128].sum(axis=1)                   # [192]
    s1 = corr_a + 2.0 * b5 / a5
    s1a = s1[:128].reshape(128, 1)
    s1b = s1[128:].reshape(64, 1)
    # odd images: A4a is {0,2} so the -sum(w) correction vanishes
    s1o = 2.0 * b5 / a5
    s1ao = s1o[:128].reshape(128, 1)
    s1bo = s1o[128:].reshape(64, 1)

    fc_w = np.asarray(inputs["fc_w"], f32)
    c6w = np.asarray(inputs["conv6_w"], f32)[:, :, 0, 0]   # [12, 192]
    Wp = (fc_w @ c6w) / np.float32(NPOS)               # [12, 192]
    Wp = Wp * (a5 / 2.0)[None, :]
    cvec = fc_w @ np.asarray(inputs["conv6_b"], f32) + np.asarray(
        inputs["fc_b"], f32)                           # [12]
    wta = np.ascontiguousarray(Wp[:, :128].T)          # [128, 12]
    wtb = np.zeros((65, 12), f32)
    wtb[:64] = Wp[:, 128:].T
    wtb[64] = cvec

    shared = {
        "w1t4": w1t4.astype(f32),
        "w2d1": w2d1.reshape(128, 256).astype(fp8),
        "w2d2": w2d2.reshape(128, 256).astype(fp8),
        "w2d3": w2d3.reshape(64, 256).astype(fp8),
        "w3d": w3d.reshape(128, 768).astype(fp8),
        "w3d3": w3d3.reshape(128, 256).astype(fp8),
        "w3s2": w3s2.reshape(128, 256).astype(fp8),
        "w4da": w4da.reshape(128, 768).astype(fp8),
        "w4d3a": w4d3a.reshape(128, 256).astype(fp8),
        "w4sa2": w4sa2.reshape(128, 256).astype(fp8),
        "w4db": w4db.reshape(128, 384).astype(fp8),
        "w4d3b": w4d3b.reshape(128, 128).astype(fp8),
        "w4sb2": w4sb2.reshape(128, 128).astype(fp8),
        "w5da": w5da.reshape(128, 256).astype(fp8),
        "w5db": w5db.reshape(128, 128).astype(fp8),
        "t1": t1.astype(f32), "nt1": (-t1).astype(f32),
        "nt2": nt2.astype(f32), "nt2p": nt2p.astype(f32),
        "t3": t3.astype(f32),
        "nt4a": nt4a.astype(f32),
        "t4ad": (-nt4a).astype(f32), "t4b": t4b.astype(f32),
        "s1a": s1a.astype(f32), "s1b": s1b.astype(f32),
        "s1ao": s1ao.astype(f32), "s1bo": s1bo.astype(f32),
        "wta": wta.astype(f32), "wtb": wtb.astype(f32),
    }
    # host im2col: cols[b, 3*ky+kx, y*64+x] = xpad[b, 2y+ky, 2x+kx];
    # split-precision blocks [X_hi; X_lo; X_hi] (see dX comment in _build)
    xpad = np.pad(x[:, 0], ((0, 0), (1, 1), (1, 1)))
    cols = np.stack([xpad[:, ky:ky + 127:2, kx:kx + 127:2]
                     for ky in range(3) for kx in range(3)],
                    axis=1).reshape(x.shape[0], 9, 4096)
    chi = rnd10(cols)
    clo = rnd10(cols - chi)
    cols27 = np.concatenate([chi, clo, chi], axis=1)   # [B*8, 27, 4096]
    in_maps = []
    for c in range(N_CORES):
        m = dict(shared)
        m["x"] = np.ascontiguousarray(cols27[c * B:(c + 1) * B])
        in_maps.append(m)
    return in_maps


def kernel(**inputs):
    cache = _build()
    in_maps = _host_prep(inputs)
    res = run_bass_kernel_spmd(cache["nc1"], in_maps,
                               core_ids=list(range(N_CORES)))
    _CACHE["last_results"] = res
    return np.concatenate([res.results[c]["y"] for c in range(N_CORES)],
                          axis=0)
